# revision 1
# baseline (speedup 1.0000x reference)
"""BERT-base forward (B=16, S=512, D=768, H=12, L=12) on 8 Trainium2 NeuronCores.

Sharding: data-parallel over batch — each core runs 2 sequences (1024 tokens)
with a full replica of the weights. No collectives.

Device layout: "transposed activations" — activations live as x^T [D, tokens]
(features on SBUF partitions, tokens on the free dim), so every linear layer is
out^T = W^T.T @ x^T with the (host-pre-tiled) weight as the stationary operand
and 512-token chunks as the moving operand.

Precision: bf16 weights (fast weight load, half DMA), fp32r residual stream
used directly as the GEMM moving operand (PE truncates to ~fp22), fp32 PSUM
accumulation.

Key structure (v2, rewritten for PE occupancy):
- Single fp32r residual stream xT; no bf16 shadow. LayerNorm biases are folded
  into downstream GEMM biases (host) and into the residual-add bias (host), so
  the device LN is only center+scale: 2 elementwise passes, split DVE/GpSimd.
- The two 512-token chunks (= the 2 sequences) are software-pipelined through
  the layer stages so each chunk's LN / softmax chains overlap the other
  chunk's GEMMs.
- Softmax denominator is folded into the attn@V matmul: the stationary is
  [V_e^T | ones] so u and den come out of one PSUM; normalize is 2 DVE ops.
- QKV bias+copy moved from ACT to DVE (tensor_scalar from PSUM).
- V^T tiles produced by rectangular DMA transposes spread across the sync and
  scalar queues, issued right after the V GEMM of each pair.
- QK^T logits for the two heads of a pair are issued interleaved so they run
  on disjoint PE row-groups (contraction=64) concurrently.
"""
import sys
sys.path.insert(0, '/opt/trn_rl_repo')

import numpy as np
import ml_dtypes
import concourse.bass as bass
import concourse.tile as tile
from concourse import bacc, mybir
from concourse.bass_utils import run_bass_kernel_spmd

# Model shapes (hardcoded)
V = 30522
S = 512
D = 768
H = 12
L = 12
F = 3072
B = 16
HD = 64
EPS = 1e-12
SCALE = HD ** (-0.5)

NCORES = 8
B_LOC = B // NCORES          # 2 sequences per core
T = B_LOC * S                # 1024 tokens per core
KD = D // 128                # 6 k-tiles over D
QKVM = 3 * D // 128          # 18 m-tiles for qkv
FM = F // 128                # 24 m-tiles over mlp hidden
PAIRS = H // 2               # 6 head-pairs
NT = B_LOC                   # 2 chunks = the 2 sequences

F32 = mybir.dt.float32
F32R = mybir.dt.float32r
BF16 = mybir.dt.bfloat16
I32 = mybir.dt.int32
AF = mybir.ActivationFunctionType
OP = mybir.AluOpType

_CACHED_NC = None


def _host_tile_weight(w_t):
    """w_t: [dout, din] torch-Linear weight. Returns [m_tiles, 128, din] where
    slice [m] is (w_t.T)[:, m*128:(m+1)*128] laid out partition-major."""
    dout, din = w_t.shape
    m_tiles, k_tiles = dout // 128, din // 128
    a = np.ascontiguousarray(w_t.T)                      # [din, dout]
    a = a.reshape(k_tiles, 128, m_tiles, 128)            # [k, p, m, w]
    a = np.ascontiguousarray(a.transpose(2, 1, 0, 3))    # [m, p, k, w]
    return a.reshape(m_tiles, 128, din).astype(ml_dtypes.bfloat16)


def build_nc(n_layers=L):
    nc = bacc.Bacc("TRN2", target_bir_lowering=False, debug=False)

    def din(name, shape, dt=BF16):
        return nc.dram_tensor(name, shape, dt, kind="ExternalInput").ap()

    nl = max(1, n_layers)
    tokens = din("tokens", [T], I32)
    tok_emb = din("tok_emb", [V, D], F32R)
    possent = din("possent", [S, D], F32)
    embw = din("embw", [D], F32)
    wqkv = din("wqkv", [nl, QKVM, 128, D])
    wvT = din("wvT", [nl, PAIRS, 128, KD, 128])
    bqkv = din("bqkv", [nl, 3 * D], F32)
    wproj = din("wproj", [nl, KD, 128, D])
    bproj = din("bproj", [nl, D], F32)          # pre-folded: + prev ln bias
    w1 = din("w1", [nl, FM, 128, D])
    b1 = din("b1", [nl, F], F32)
    w2 = din("w2", [nl, KD, 128, F])
    b2 = din("b2", [nl, D], F32)                # pre-folded: + ln1 bias
    ln1w = din("ln1w", [nl, D], F32)
    ln2w = din("ln2w", [nl, D], F32)
    wpool = din("wpool", [KD, 128, D], BF16)
    bpool = din("bpool", [D], F32)
    ident = din("ident", [128, 128], F32R)
    ones = din("ones", [128, 128], F32R)
    out = nc.dram_tensor("out", [B_LOC, D], F32, kind="ExternalOutput").ap()

    with tile.TileContext(nc) as tc:
        _build_body(nc, tc, n_layers, tokens, tok_emb, possent, embw,
                    wqkv, wvT, bqkv, wproj, bproj, w1, b1, w2, b2,
                    ln1w, ln2w, wpool, bpool, ident, ones, out)
    nc.compile()
    return nc


def _build_body(nc, tc, n_layers, tokens, tok_emb, possent, embw,
                wqkv, wvT, bqkv, wproj, bproj, w1, b1, w2, b2,
                ln1w, ln2w, wpool, bpool, ident, ones, out):
    from contextlib import ExitStack
    ctx = ExitStack()
    with ctx:
        consts = ctx.enter_context(tc.tile_pool(name="consts", bufs=1))
        xpool = ctx.enter_context(tc.tile_pool(name="xpool", bufs=1))
        qkvpool = ctx.enter_context(tc.tile_pool(name="qkvpool", bufs=6))
        expool = ctx.enter_context(tc.tile_pool(name="expool", bufs=4))
        wpool6 = ctx.enter_context(tc.tile_pool(name="wpool6", bufs=8))
        wpool24 = ctx.enter_context(tc.tile_pool(name="wpool24", bufs=2))
        biasp = ctx.enter_context(tc.tile_pool(name="biasp", bufs=2))
        lnp = ctx.enter_context(tc.tile_pool(name="lnp", bufs=6))
        psum = ctx.enter_context(tc.tile_pool(name="psum", bufs=8, space="PSUM"))

        ident_sb = consts.tile([128, 128], F32R)
        nc.sync.dma_start(ident_sb[:], ident)
        ones_sb = consts.tile([128, 128], F32R)
        nc.sync.dma_start(ones_sb[:], ones)
        eps_sb = consts.tile([128, 1], F32)
        nc.vector.memset(eps_sb[:], EPS)
        onesb_sb = consts.tile([128, 128], BF16)
        nc.gpsimd.memset(onesb_sb[:], 1.0)

        # residual stream x^T (fp32) + bf16 shadow for GEMM moving operands
        # (the HW matmul cannot mix f32r rhs with bf16 weights)
        xT = xpool.tile([128, KD, T], F32R)
        xTb = xpool.tile([128, KD, T], BF16)
        # attention output a^T (bf16: proj GEMM input)
        aT = xpool.tile([128, KD, T], BF16)
        # MLP hidden per chunk (bf16)
        hT = [xpool.tile([128, FM, S], BF16, name=f"hT{s}") for s in range(NT)]
        # static [V_e^T | ones] stationaries: [parity][e] -> [128, 4, 128]
        # ones half initialized once; transposes overwrite only the V half.
        vts = [[consts.tile([128, 4, 128], BF16, name=f"vts{p}{e}")
                for e in range(2)] for p in range(2)]
        for p in range(2):
            nc.gpsimd.memset(vts[p][0][:, :, 64:128], 1.0)
            nc.gpsimd.memset(vts[p][1][:, :, 0:64], 1.0)

        def ps_mm(shape=None, tag="mm"):
            return psum.tile(shape or [128, S], F32, tag="mm", name="ps")

        # ---------------- Embedding ----------------
        embp = ctx.enter_context(tc.tile_pool(name="embp", bufs=2))
        embw_sb = embp.tile([128, D], F32, tag="embw", bufs=1)
        nc.sync.dma_start(embw_sb[:], embw[None, :].to_broadcast([128, D]))

        def embed_tt(tt):
            idx_sb = embp.tile([128, 1], I32, tag="idx")
            nc.sync.dma_start(idx_sb[:], tokens[tt * 128:(tt + 1) * 128, None])
            g_sb = embp.tile([128, D], F32R, tag="g")
            nc.gpsimd.indirect_dma_start(
                out=g_sb[:], out_offset=None, in_=tok_emb,
                in_offset=bass.IndirectOffsetOnAxis(ap=idx_sb[:, :1], axis=0))
            p_sb = embp.tile([128, D], F32, tag="p")
            prow = (tt * 128) % S
            nc.sync.dma_start(p_sb[:], possent[prow:prow + 128, :])
            nc.vector.tensor_add(g_sb[:], g_sb[:], p_sb[:])
            # LayerNorm over free dim (d): bn_stats in 2 subgroups of 384
            st_sb = embp.tile([128, 2, 6], F32, tag="st")
            gv = g_sb[:].rearrange("p (a b) -> p a b", a=2)
            for a in range(2):
                nc.vector.bn_stats(st_sb[:, a, :], gv[:, a, :])
            mv = embp.tile([128, 2], F32, tag="mv")
            nc.vector.bn_aggr(mv[:], st_sb[:])
            sd = embp.tile([128, 1], F32, tag="sd")
            nc.scalar.activation(sd[:], mv[:, 1:2], AF.Sqrt, bias=eps_sb[:])
            nc.vector.reciprocal_approx_fast(sd[:], sd[:])
            nc.vector.tensor_scalar(g_sb[:], g_sb[:], mv[:, 0:1], sd[:],
                                    op0=OP.subtract, op1=OP.mult)
            nc.vector.tensor_mul(g_sb[:], g_sb[:], embw_sb[:])
            # transpose into xT (no ln bias: folded into l=0 qkv bias and
            # the l=0 proj residual-add bias on the host)
            for k in range(KD):
                pst = psum.tile([128, 128], F32R, tag="mm", name="pst")
                nc.tensor.transpose(pst[:], g_sb[:, k * 128:(k + 1) * 128],
                                    ident_sb[:])
                ttsl = slice(tt * 128, (tt + 1) * 128)
                nc.vector.tensor_copy(xT[:, k, ttsl], pst[:])
                nc.gpsimd.tensor_copy(xTb[:, k, ttsl], xT[:, k, ttsl])

        # chunk A now; chunk B (tt 4..7) deferred into layer-0 attention hooks
        for tt in range(4):
            embed_tt(tt)

        # ---------------- Per-layer helpers ----------------
        def load_layer_bias(l):
            bq_sb = biasp.tile([128, QKVM], F32, tag="bq")
            nc.sync.dma_start(bq_sb[:], bqkv[l].rearrange("(m p) -> p m", p=128))
            bp_sb = biasp.tile([128, KD], F32, tag="bp")
            nc.sync.dma_start(bp_sb[:], bproj[l].rearrange("(m p) -> p m", p=128))
            b1_sb = biasp.tile([128, FM], F32, tag="b1")
            nc.sync.dma_start(b1_sb[:], b1[l].rearrange("(m p) -> p m", p=128))
            b2_sb = biasp.tile([128, KD], F32, tag="b2")
            nc.sync.dma_start(b2_sb[:], b2[l].rearrange("(m p) -> p m", p=128))
            l1w_sb = biasp.tile([128, KD], F32, tag="l1w")
            nc.sync.dma_start(l1w_sb[:], ln1w[l].rearrange("(k p) -> p k", p=128))
            l2w_sb = biasp.tile([128, KD], F32, tag="l2w")
            nc.sync.dma_start(l2w_sb[:], ln2w[l].rearrange("(k p) -> p k", p=128))
            return dict(bq=bq_sb, bp=bp_sb, b1=b1_sb, b2=b2_sb,
                        l1w=l1w_sb, l2w=l2w_sb)

        # per-(chunk, pair) state: q/k/v tiles + expP tiles
        attn_st = {}

        def qkv_pair(l, s, pr, bias):
            """V^T, K, Q GEMMs for pair pr of chunk s + logits + exp.

            V is computed directly in [token, feature] layout: stationary =
            xTb token-slices, moving = Wv^T k-tiles, so no transposes are
            needed. The V bias is folded into the proj bias on the host
            (softmax rows sum to 1, so it shifts attn output by a constant).
            """
            tsl = slice(s * S, (s + 1) * S)
            p2 = pr % 2
            wv_sb = wpool6.tile([128, D], BF16, tag="w6", name="wv_sb")
            nc.sync.dma_start(wv_sb[:], wvT[l, pr].rearrange("p k f -> p (k f)"))
            wvv = wv_sb[:].rearrange("p (k f) -> p k f", k=KD)
            for st in range(4):
                ksl = slice(s * S + st * 128, s * S + (st + 1) * 128)
                psv = ps_mm([128, 128], tag="psv")
                for k in range(KD):
                    nc.tensor.matmul(psv[:], lhsT=xTb[:, k, ksl],
                                     rhs=wvv[:, k, :],
                                     start=(k == 0), stop=(k == KD - 1))
                nc.vector.tensor_copy(vts[p2][0][:, st, 0:64], psv[:, 0:64])
                nc.vector.tensor_copy(vts[p2][1][:, st, 64:128], psv[:, 64:128])
            tiles = {}
            for mi, m in ((1, 6 + pr), (0, pr)):
                w_sb = wpool6.tile([128, D], BF16, tag="w6")
                nc.sync.dma_start(w_sb[:], wqkv[l, m])
                ps = ps_mm()
                for k in range(KD):
                    nc.tensor.matmul(ps[:], lhsT=w_sb[:, k * 128:(k + 1) * 128],
                                     rhs=xTb[:, k, tsl],
                                     start=(k == 0), stop=(k == KD - 1))
                t_sb = qkvpool.tile([128, S], BF16, tag="qkv")
                if mi == 0:
                    # Q keeps its bias; K's bias only shifts logits by a
                    # per-query constant, which softmax ignores.
                    nc.vector.tensor_scalar(t_sb[:], ps[:],
                                            bias['bq'][:, m:m + 1],
                                            None, op0=OP.add)
                else:
                    nc.vector.tensor_copy(t_sb[:], ps[:])
                tiles[mi] = t_sb
            qt, kt = tiles[0], tiles[1]
            # logits + exp, heads interleaved for PE row-group concurrency
            expP = [expool.tile([128, 4, S], BF16, tag="expP", name="expP")
                    for _ in range(2)]
            for st in range(4):
                for e in range(2):
                    po = 64 * e
                    psl = ps_mm()
                    nc.tensor.matmul(
                        psl[:],
                        lhsT=kt[po:po + 64, st * 128:(st + 1) * 128],
                        rhs=qt[po:po + 64, :],
                        start=True, stop=True)
                    nc.scalar.activation(expP[e][:, st, :], psl[:],
                                         AF.Exp, scale=SCALE)
            attn_st[(s, pr)] = expP

        def av_pair(l, s, pr):
            """attn@V with folded denominator + DVE normalize into aT."""
            tsl = slice(s * S, (s + 1) * S)
            expP = attn_st.pop((s, pr))
            p2 = pr % 2
            psA = ps_mm()
            psB = ps_mm()
            for st in range(4):
                nc.tensor.matmul(psA[:], lhsT=vts[p2][0][:, st, :],
                                 rhs=expP[0][:, st, :],
                                 start=(st == 0), stop=(st == 3))
                nc.tensor.matmul(psB[:], lhsT=vts[p2][1][:, st, :],
                                 rhs=expP[1][:, st, :],
                                 start=(st == 0), stop=(st == 3))
            # DVE operands must share base partition, and the reciprocal
            # custom op only works at base 0 (HW rules): hop den0 down via a
            # small SB->SB DMA, reciprocal at base 0, multiply same-base.
            recA = lnp.tile([128, S], F32, tag="rec", name="recA", bufs=2)
            nc.vector.tensor_copy(recA[64:128, :], psA[64:128, :])
            nc.gpsimd.dma_start(recA[0:64, :], recA[64:128, :])
            nc.vector.reciprocal_approx_fast(recA[0:64, :], recA[0:64, :])
            nc.vector.tensor_mul(aT[0:64, pr, tsl], psA[0:64, :], recA[0:64, :])
            recB = lnp.tile([128, S], F32, tag="rec", name="recB", bufs=2)
            nc.vector.reciprocal_approx_fast(recB[0:64, :], psB[0:64, :])
            nc.gpsimd.dma_start(recB[64:128, :], recB[0:64, :])
            nc.vector.tensor_mul(aT[64:128, pr, tsl], psB[64:128, :],
                                 recB[64:128, :])

        def proj_chunk(l, s, bias):
            tsl = slice(s * S, (s + 1) * S)
            for m in range(KD):
                w_sb = wpool6.tile([128, D], BF16, tag="w6")
                nc.sync.dma_start(w_sb[:], wproj[l, m])
                ps = ps_mm()
                for k in range(KD):
                    nc.tensor.matmul(ps[:], lhsT=w_sb[:, k * 128:(k + 1) * 128],
                                     rhs=aT[:, k, tsl],
                                     start=(k == 0), stop=(k == KD - 1))
                nc.vector.scalar_tensor_tensor(
                    xT[:, m, tsl], in0=ps[:], scalar=bias['bp'][:, m:m + 1],
                    in1=xT[:, m, tsl], op0=OP.add, op1=OP.add)

        def stats_chunk(s):
            """LN stats matmuls on chunk s. Returns (ps_s, ps_q) PSUM tiles.

            Squares are split across DVE and GpSimd (bf16 out) so the ps_q
            accumulation is not serialized behind one engine."""
            tsl = slice(s * S, (s + 1) * S)
            sqs = []
            for k in range(KD):
                sq = lnp.tile([128, S], BF16, tag="sq")
                eng = nc.vector if k % 2 == 0 else nc.gpsimd
                eng.tensor_mul(sq[:], xT[:, k, tsl], xT[:, k, tsl])
                sqs.append(sq)
            ps_s = ps_mm()
            for k in range(KD):
                nc.tensor.matmul(ps_s[:], lhsT=ones_sb[:], rhs=xT[:, k, tsl],
                                 start=(k == 0), stop=(k == KD - 1))
            ps_q = ps_mm()
            for k in range(KD):
                nc.tensor.matmul(ps_q[:], lhsT=onesb_sb[:], rhs=sqs[k][:],
                                 start=(k == 0), stop=(k == KD - 1))
            return ps_s, ps_q

        def ln_chain(s, ps_s, ps_q, lw_sb):
            """Center+scale xT chunk s in place (2 passes/k, DVE+gp split)."""
            tsl = slice(s * S, (s + 1) * S)
            mean = lnp.tile([128, S], F32, tag="mean", bufs=2)
            nc.vector.tensor_scalar_mul(mean[:], ps_s[:], 1.0 / D)
            var = lnp.tile([128, S], F32, tag="var", bufs=2)
            nc.vector.tensor_mul(var[:], mean[:], mean[:])
            nc.vector.scalar_tensor_tensor(var[:], in0=ps_q[:], scalar=1.0 / D,
                                           in1=var[:], op0=OP.mult,
                                           op1=OP.subtract)
            nc.scalar.activation(var[:], var[:], AF.Sqrt, bias=eps_sb[:])
            r = lnp.tile([128, S], F32, tag="r", bufs=2)
            nc.vector.reciprocal_approx_fast(r[:], var[:])
            for k in range(KD):
                xk = xT[:, k, tsl]
                nc.gpsimd.tensor_sub(xk, xk, mean[:])
                nc.vector.scalar_tensor_tensor(xk, in0=xk,
                                               scalar=lw_sb[:, k:k + 1],
                                               in1=r[:], op0=OP.mult,
                                               op1=OP.mult)
                nc.gpsimd.tensor_copy(xTb[:, k, tsl], xk)

        def mlp1_chunk(l, s, bias, hooks=None):
            hooks = hooks or {}
            tsl = slice(s * S, (s + 1) * S)
            for m in range(FM):
                w_sb = wpool6.tile([128, D], BF16, tag="w6")
                nc.sync.dma_start(w_sb[:], w1[l, m])
                ps = ps_mm()
                for k in range(KD):
                    nc.tensor.matmul(ps[:], lhsT=w_sb[:, k * 128:(k + 1) * 128],
                                     rhs=xTb[:, k, tsl],
                                     start=(k == 0), stop=(k == KD - 1))
                nc.scalar.activation(hT[s][:, m, :], ps[:], AF.Gelu,
                                     bias=bias['b1'][:, m:m + 1])
                if m in hooks:
                    hooks[m]()

        def mlp2_chunk(l, s, bias):
            tsl = slice(s * S, (s + 1) * S)
            for m in range(KD):
                w_sb = wpool24.tile([128, F], BF16, tag="w24")
                nc.sync.dma_start(w_sb[:], w2[l, m])
                ps = ps_mm()
                for k in range(FM):
                    nc.tensor.matmul(ps[:], lhsT=w_sb[:, k * 128:(k + 1) * 128],
                                     rhs=hT[s][:, k, :],
                                     start=(k == 0), stop=(k == FM - 1))
                nc.vector.scalar_tensor_tensor(
                    xT[:, m, tsl], in0=ps[:], scalar=bias['b2'][:, m:m + 1],
                    in1=xT[:, m, tsl], op0=OP.add, op1=OP.add)

        def attn_block(l, s, bias, hooks=None):
            hooks = hooks or {}
            for pr in range(PAIRS):
                qkv_pair(l, s, pr, bias)
                if pr in hooks:
                    hooks[pr]()
                if pr >= 1:
                    av_pair(l, s, pr - 1)
            av_pair(l, s, PAIRS - 1)

        # ---------------- Pooler (per-seq, so seq0 runs under MLP cover) ----
        poolp = ctx.enter_context(tc.tile_pool(name="poolp", bufs=1))
        bpl_sb = poolp.tile([128, KD], F32)
        nc.sync.dma_start(bpl_sb[:], bpool.rearrange("(m p) -> p m", p=128))
        pool_sb = poolp.tile([128, KD, B_LOC], F32R)
        poolw_sb = [poolp.tile([128, D], BF16, name=f"poolw{m}")
                    for m in range(KD)]
        for m in range(KD):
            nc.sync.dma_start(poolw_sb[m][:], wpool[m])

        def pooler_seq(s):
            for m in range(KD):
                ps = ps_mm([128, 1])
                for k in range(KD):
                    nc.tensor.matmul(ps[:], lhsT=poolw_sb[m][:, k * 128:(k + 1) * 128],
                                     rhs=xTb[:, k, s * S:s * S + 1],
                                     start=(k == 0), stop=(k == KD - 1))
                nc.scalar.activation(pool_sb[:, m, s:s + 1], ps[:], AF.Tanh,
                                     bias=bpl_sb[:, m:m + 1])

        def pooler_out():
            out_sb = poolp.tile([128, D], F32)
            for k in range(KD):
                pst = psum.tile([128, 128], F32R, tag="mm", name="pst")
                nc.tensor.transpose(pst[:B_LOC, :], pool_sb[:, k, :], ident_sb[:])
                nc.vector.tensor_copy(out_sb[:B_LOC, k * 128:(k + 1) * 128],
                                      pst[:B_LOC, :])
            nc.sync.dma_start(out, out_sb[:B_LOC, :])

        # ---------------- Layers (2-chunk software pipeline) ----------------
        # chunk B's final LN2 of layer l-1 is emitted at the start of layer
        # l's chunk-A attention (one DVE burst per attention block); stats
        # matmuls run right after their producer GEMM stage.
        pending = None  # (ps_s, ps_q, lw) for chunk B's LN2 of prev layer
        for l in range(n_layers):
            bias = load_layer_bias(l)
            if l == 0:
                hooksA = {pr: (lambda tt=tt: embed_tt(tt))
                          for pr, tt in zip(range(4), range(4, 8))}
            else:
                p = pending
                hooksA = {0: (lambda p=p: ln_chain(1, p[0], p[1], p[2]))}
            attn_block(l, 0, bias, hooks=hooksA)
            proj_chunk(l, 0, bias)
            sA = stats_chunk(0)
            attn_block(l, 1, bias,
                       hooks={0: (lambda sA=sA, bias=bias:
                                  ln_chain(0, sA[0], sA[1], bias['l1w']))})
            proj_chunk(l, 1, bias)
            sB = stats_chunk(1)
            mlp1_chunk(l, 0, bias,
                       hooks={2: (lambda sB=sB, bias=bias:
                                  ln_chain(1, sB[0], sB[1], bias['l1w']))})
            mlp2_chunk(l, 0, bias)
            s2A = stats_chunk(0)
            hooksB1 = {2: (lambda s2A=s2A, bias=bias:
                           ln_chain(0, s2A[0], s2A[1], bias['l2w']))}
            if l == n_layers - 1:
                hooksB1[10] = lambda: pooler_seq(0)
            mlp1_chunk(l, 1, bias, hooks=hooksB1)
            mlp2_chunk(l, 1, bias)
            s2B = stats_chunk(1)
            pending = (s2B[0], s2B[1], bias['l2w'])
        ln_chain(1, pending[0], pending[1], pending[2])
        pooler_seq(1)
        pooler_out()




def _prep_host(inputs, n_layers=L):
    f32 = lambda a: np.asarray(a, dtype=np.float32)
    tokens = np.asarray(inputs["tokens"]).astype(np.int32)          # [16, 512]
    possent = f32(inputs["pos_emb"])[0] + f32(inputs["sent_emb"])[0, 0][None, :]

    def tile_stack(w, n):  # w: [L, dout, din]
        n = max(1, n)
        return np.stack([_host_tile_weight(f32(w[i])) for i in range(n)])

    nl = max(1, n_layers)
    # The residual stream xT holds LN output WITHOUT the ln bias. Folds:
    #   qkv_b[l]  += qkv_w[l]  @ prev_ln_b   (emb_ln_b for l=0, ln2_b[l-1] else)
    #   mlp_b1[l] += mlp_w1[l] @ ln1_b[l]
    #   pool_b    += pool_w    @ ln2_b[last]
    #   proj_b[l] += prev_ln_b               (residual-add absorbs the bias)
    #   mlp_b2[l] += ln1_b[l]
    qkv_b = f32(inputs["qkv_b"]).copy()
    proj_b = f32(inputs["proj_b"]).copy()
    mlp_b1 = f32(inputs["mlp_b1"]).copy()
    mlp_b2 = f32(inputs["mlp_b2"]).copy()
    pool_b = f32(inputs["pool_b"]).copy()
    emb_ln_b = f32(inputs["emb_ln_b"])
    ln1_b = f32(inputs["ln1_b"])
    ln2_b = f32(inputs["ln2_b"])
    wvT = np.empty((nl, PAIRS, 128, KD, 128), dtype=ml_dtypes.bfloat16)
    for l in range(nl):
        prev_b = emb_ln_b if l == 0 else ln2_b[l - 1]
        qkv_b[l] = qkv_b[l] + f32(inputs["qkv_w"][l]) @ prev_b
        # the (folded) V bias shifts attn output by a constant: fold into
        # the proj bias instead of applying it on device
        bv = qkv_b[l][2 * D:3 * D].copy()
        qkv_b[l][2 * D:3 * D] = 0.0
        proj_b[l] = proj_b[l] + prev_b + f32(inputs["proj_w"][l]) @ bv
        mlp_b1[l] = mlp_b1[l] + f32(inputs["mlp_w1"][l]) @ ln1_b[l]
        mlp_b2[l] = mlp_b2[l] + ln1_b[l]
        for pr in range(PAIRS):
            vw = f32(inputs["qkv_w"][l])[2 * D + 128 * pr:2 * D + 128 * (pr + 1), :]
            wvT[l, pr] = vw.T.reshape(KD, 128, 128).transpose(1, 0, 2)
    if n_layers >= 1:
        pool_b = pool_b + f32(inputs["pool_w"]) @ ln2_b[nl - 1]
    else:
        pool_b = pool_b + f32(inputs["pool_w"]) @ emb_ln_b
    common = {
        "tok_emb": f32(inputs["tok_emb"]),
        "possent": possent.astype(np.float32),
        "embw": f32(inputs["emb_ln_w"]),
        "wqkv": tile_stack(inputs["qkv_w"], n_layers),
        "wvT": wvT,
        "bqkv": qkv_b[:nl],
        "wproj": tile_stack(inputs["proj_w"], n_layers),
        "bproj": proj_b[:nl],
        "w1": tile_stack(inputs["mlp_w1"], n_layers),
        "b1": mlp_b1[:nl],
        "w2": tile_stack(inputs["mlp_w2"], n_layers),
        "b2": mlp_b2[:nl],
        "ln1w": f32(inputs["ln1_w"])[:nl],
        "ln2w": f32(inputs["ln2_w"])[:nl],
        "wpool": _host_tile_weight(f32(inputs["pool_w"])),
        "bpool": pool_b,
        "ident": np.eye(128, dtype=np.float32),
        "ones": np.ones((128, 128), dtype=np.float32),
    }
    in_maps = []
    for c in range(NCORES):
        m = dict(common)
        m["tokens"] = np.ascontiguousarray(
            tokens[c * B_LOC:(c + 1) * B_LOC].reshape(-1))
        in_maps.append(m)
    return in_maps


def kernel(**inputs) -> np.ndarray:
    global _CACHED_NC
    if _CACHED_NC is None:
        _CACHED_NC = build_nc(L)
    in_maps = _prep_host(inputs, L)
    res = run_bass_kernel_spmd(_CACHED_NC, in_maps,
                               core_ids=list(range(NCORES)), trace=False)
    return np.concatenate([res.results[c]["out"] for c in range(NCORES)], axis=0)



# revision 21
# speedup vs baseline: 1.0583x; 1.0583x over previous
"""BERT-base forward (B=16, S=512, D=768, H=12, L=12) on 8 Trainium2 NeuronCores.

Sharding: data-parallel over batch — each core runs 2 sequences (1024 tokens)
with a full replica of the weights. No collectives.

Device layout: "transposed activations" — activations live as x^T [D, tokens]
(features on SBUF partitions, tokens on the free dim), so every linear layer is
out^T = W^T.T @ x^T with the (host-pre-tiled) weight as the stationary operand
and 512-token chunks as the moving operand.

Precision (v3): fp16 weights and fp16 residual stream, used directly as the
GEMM moving operand (no bf16 shadow copy), fp32 PSUM accumulation. fp16 LN
mean/r operands enable 2x DVE perf modes; Q/K PSUM->SBUF copies moved to the
scalar engine.

Key structure (v2, rewritten for PE occupancy):
- Single fp32r residual stream xT; no bf16 shadow. LayerNorm biases are folded
  into downstream GEMM biases (host) and into the residual-add bias (host), so
  the device LN is only center+scale: 2 elementwise passes, split DVE/GpSimd.
- The two 512-token chunks (= the 2 sequences) are software-pipelined through
  the layer stages so each chunk's LN / softmax chains overlap the other
  chunk's GEMMs.
- Softmax denominator is folded into the attn@V matmul: the stationary is
  [V_e^T | ones] so u and den come out of one PSUM; normalize is 2 DVE ops.
- QKV bias+copy moved from ACT to DVE (tensor_scalar from PSUM).
- V^T tiles produced by rectangular DMA transposes spread across the sync and
  scalar queues, issued right after the V GEMM of each pair.
- QK^T logits for the two heads of a pair are issued interleaved so they run
  on disjoint PE row-groups (contraction=64) concurrently.
"""
import sys
sys.path.insert(0, '/opt/trn_rl_repo')

import numpy as np
import ml_dtypes
import concourse.bass as bass
import concourse.tile as tile
from concourse import bacc, mybir
from concourse.bass_utils import run_bass_kernel_spmd

# Model shapes (hardcoded)
V = 30522
S = 512
D = 768
H = 12
L = 12
F = 3072
B = 16
HD = 64
EPS = 1e-12
SCALE = HD ** (-0.5)

NCORES = 8
B_LOC = B // NCORES          # 2 sequences per core
T = B_LOC * S                # 1024 tokens per core
KD = D // 128                # 6 k-tiles over D
QKVM = 3 * D // 128          # 18 m-tiles for qkv
FM = F // 128                # 24 m-tiles over mlp hidden
PAIRS = H // 2               # 6 head-pairs
NT = B_LOC                   # 2 chunks = the 2 sequences

F32 = mybir.dt.float32
F32R = mybir.dt.float32r
BF16 = mybir.dt.bfloat16
F16 = mybir.dt.float16
I32 = mybir.dt.int32
AF = mybir.ActivationFunctionType
OP = mybir.AluOpType

_CACHED_NC = None


def _host_tile_weight(w_t):
    """w_t: [dout, din] torch-Linear weight. Returns [m_tiles, 128, din] where
    slice [m] is (w_t.T)[:, m*128:(m+1)*128] laid out partition-major."""
    dout, din = w_t.shape
    m_tiles, k_tiles = dout // 128, din // 128
    a = np.ascontiguousarray(w_t.T)                      # [din, dout]
    a = a.reshape(k_tiles, 128, m_tiles, 128)            # [k, p, m, w]
    a = np.ascontiguousarray(a.transpose(2, 1, 0, 3))    # [m, p, k, w]
    return a.reshape(m_tiles, 128, din).astype(np.float16)


def build_nc(n_layers=L):
    nc = bacc.Bacc("TRN2", target_bir_lowering=False, debug=False)

    def din(name, shape, dt=F16):
        return nc.dram_tensor(name, shape, dt, kind="ExternalInput").ap()

    nl = max(1, n_layers)
    tokens = din("tokens", [T], I32)
    tok_emb = din("tok_emb", [V, D], F32R)
    possent = din("possent", [S, D], F32)
    embw = din("embw", [D], F32)
    wqkv = din("wqkv", [nl, QKVM, 128, D])
    wvT = din("wvT", [nl, PAIRS, 128, KD, 128])
    bqkv = din("bqkv", [nl, 3 * D], F32)
    wproj = din("wproj", [nl, KD, 128, D])
    bproj = din("bproj", [nl, D], F32)          # pre-folded: + prev ln bias
    w1 = din("w1", [nl, FM, 128, D])
    b1 = din("b1", [nl, F], F32)
    w2 = din("w2", [nl, KD, 128, F])
    b2 = din("b2", [nl, D], F32)                # pre-folded: + ln1 bias
    ln1w = din("ln1w", [nl, D], F32)
    ln2w = din("ln2w", [nl, D], F32)
    wpool = din("wpool", [KD, 128, D], F16)
    bpool = din("bpool", [D], F32)
    ident = din("ident", [128, 128], F32R)
    out = nc.dram_tensor("out", [B_LOC, D], F32, kind="ExternalOutput").ap()

    with tile.TileContext(nc) as tc:
        _build_body(nc, tc, n_layers, tokens, tok_emb, possent, embw,
                    wqkv, wvT, bqkv, wproj, bproj, w1, b1, w2, b2,
                    ln1w, ln2w, wpool, bpool, ident, out)
    nc.compile()
    return nc


def _build_body(nc, tc, n_layers, tokens, tok_emb, possent, embw,
                wqkv, wvT, bqkv, wproj, bproj, w1, b1, w2, b2,
                ln1w, ln2w, wpool, bpool, ident, out):
    from contextlib import ExitStack
    ctx = ExitStack()
    with ctx:
        consts = ctx.enter_context(tc.tile_pool(name="consts", bufs=1))
        xpool = ctx.enter_context(tc.tile_pool(name="xpool", bufs=1))
        qkvpool = ctx.enter_context(tc.tile_pool(name="qkvpool", bufs=6))
        expool = ctx.enter_context(tc.tile_pool(name="expool", bufs=4))
        wpool6 = ctx.enter_context(tc.tile_pool(name="wpool6", bufs=8))
        wpool24 = ctx.enter_context(tc.tile_pool(name="wpool24", bufs=2))
        biasp = ctx.enter_context(tc.tile_pool(name="biasp", bufs=2))
        lnp = ctx.enter_context(tc.tile_pool(name="lnp", bufs=6))
        psum = ctx.enter_context(tc.tile_pool(name="psum", bufs=8, space="PSUM"))

        ident_sb = consts.tile([128, 128], F32R)
        nc.sync.dma_start(ident_sb[:], ident)
        ones_sb = consts.tile([128, 128], F16)
        nc.gpsimd.memset(ones_sb[:], 1.0)
        eps_sb = consts.tile([128, 1], F32)
        nc.vector.memset(eps_sb[:], EPS)

        # residual stream x^T (fp16: used directly as the GEMM moving operand)
        xT = xpool.tile([128, KD, T], F16)
        # attention output a^T (fp16: proj GEMM input)
        aT = xpool.tile([128, KD, T], F16)
        # MLP hidden per chunk (fp16)
        hT = [xpool.tile([128, FM, S], F16, name=f"hT{s}") for s in range(NT)]
        # static [V_e^T | ones] stationaries: [parity][e] -> [128, 4, 128]
        # ones half initialized once; transposes overwrite only the V half.
        vts = [[consts.tile([128, 4, 128], F16, name=f"vts{p}{e}")
                for e in range(2)] for p in range(2)]
        for p in range(2):
            nc.gpsimd.memset(vts[p][0][:, :, 64:128], 1.0)
            nc.gpsimd.memset(vts[p][1][:, :, 0:64], 1.0)

        def ps_mm(shape=None, tag="mm"):
            return psum.tile(shape or [128, S], F32, tag="mm", name="ps")

        # ---------------- Embedding ----------------
        embp = ctx.enter_context(tc.tile_pool(name="embp", bufs=2))
        embw_sb = embp.tile([128, D], F32, tag="embw", bufs=1)
        nc.sync.dma_start(embw_sb[:], embw[None, :].to_broadcast([128, D]))

        def embed_tt(tt):
            idx_sb = embp.tile([128, 1], I32, tag="idx")
            nc.sync.dma_start(idx_sb[:], tokens[tt * 128:(tt + 1) * 128, None])
            g_sb = embp.tile([128, D], F32R, tag="g")
            nc.gpsimd.indirect_dma_start(
                out=g_sb[:], out_offset=None, in_=tok_emb,
                in_offset=bass.IndirectOffsetOnAxis(ap=idx_sb[:, :1], axis=0))
            p_sb = embp.tile([128, D], F32, tag="p")
            prow = (tt * 128) % S
            nc.sync.dma_start(p_sb[:], possent[prow:prow + 128, :])
            nc.vector.tensor_add(g_sb[:], g_sb[:], p_sb[:])
            # LayerNorm over free dim (d): bn_stats in 2 subgroups of 384
            st_sb = embp.tile([128, 2, 6], F32, tag="st")
            gv = g_sb[:].rearrange("p (a b) -> p a b", a=2)
            for a in range(2):
                nc.vector.bn_stats(st_sb[:, a, :], gv[:, a, :])
            mv = embp.tile([128, 2], F32, tag="mv")
            nc.vector.bn_aggr(mv[:], st_sb[:])
            sd = embp.tile([128, 1], F32, tag="sd")
            nc.scalar.activation(sd[:], mv[:, 1:2], AF.Sqrt, bias=eps_sb[:])
            nc.vector.reciprocal_approx_fast(sd[:], sd[:])
            nc.vector.tensor_scalar(g_sb[:], g_sb[:], mv[:, 0:1], sd[:],
                                    op0=OP.subtract, op1=OP.mult)
            nc.vector.tensor_mul(g_sb[:], g_sb[:], embw_sb[:])
            # transpose into xT (no ln bias: folded into l=0 qkv bias and
            # the l=0 proj residual-add bias on the host)
            for k in range(KD):
                pst = psum.tile([128, 128], F32R, tag="mm", name="pst")
                nc.tensor.transpose(pst[:], g_sb[:, k * 128:(k + 1) * 128],
                                    ident_sb[:])
                ttsl = slice(tt * 128, (tt + 1) * 128)
                nc.vector.tensor_copy(xT[:, k, ttsl], pst[:])

        # chunk A now; chunk B (tt 4..7) deferred into layer-0 attention hooks
        for tt in range(4):
            embed_tt(tt)

        # ---------------- Per-layer helpers ----------------
        def load_layer_bias(l):
            bq_sb = biasp.tile([128, QKVM], F32, tag="bq")
            nc.sync.dma_start(bq_sb[:], bqkv[l].rearrange("(m p) -> p m", p=128))
            bp_sb = biasp.tile([128, KD], F32, tag="bp")
            nc.sync.dma_start(bp_sb[:], bproj[l].rearrange("(m p) -> p m", p=128))
            b1_sb = biasp.tile([128, FM], F32, tag="b1")
            nc.sync.dma_start(b1_sb[:], b1[l].rearrange("(m p) -> p m", p=128))
            b2_sb = biasp.tile([128, KD], F32, tag="b2")
            nc.sync.dma_start(b2_sb[:], b2[l].rearrange("(m p) -> p m", p=128))
            l1w_sb = biasp.tile([128, KD], F32, tag="l1w")
            nc.sync.dma_start(l1w_sb[:], ln1w[l].rearrange("(k p) -> p k", p=128))
            l2w_sb = biasp.tile([128, KD], F32, tag="l2w")
            nc.sync.dma_start(l2w_sb[:], ln2w[l].rearrange("(k p) -> p k", p=128))
            return dict(bq=bq_sb, bp=bp_sb, b1=b1_sb, b2=b2_sb,
                        l1w=l1w_sb, l2w=l2w_sb)

        # per-(chunk, pair) state: q/k/v tiles + expP tiles
        attn_st = {}

        def qkv_pair(l, s, pr, bias):
            """V^T, K, Q GEMMs for pair pr of chunk s + logits + exp.

            V is computed directly in [token, feature] layout: stationary =
            xTb token-slices, moving = Wv^T k-tiles, so no transposes are
            needed. The V bias is folded into the proj bias on the host
            (softmax rows sum to 1, so it shifts attn output by a constant).
            """
            tsl = slice(s * S, (s + 1) * S)
            p2 = pr % 2
            wv_sb = wpool6.tile([128, D], F16, tag="w6", name="wv_sb")
            nc.sync.dma_start(wv_sb[:], wvT[l, pr].rearrange("p k f -> p (k f)"))
            wvv = wv_sb[:].rearrange("p (k f) -> p k f", k=KD)
            for st in range(4):
                ksl = slice(s * S + st * 128, s * S + (st + 1) * 128)
                psv = ps_mm([128, 128], tag="psv")
                for k in range(KD):
                    nc.tensor.matmul(psv[:], lhsT=xT[:, k, ksl],
                                     rhs=wvv[:, k, :],
                                     start=(k == 0), stop=(k == KD - 1))
                nc.vector.tensor_copy(vts[p2][0][:, st, 0:64], psv[:, 0:64])
                nc.vector.tensor_copy(vts[p2][1][:, st, 64:128], psv[:, 64:128])
            tiles = {}
            for mi, m in ((1, 6 + pr), (0, pr)):
                w_sb = wpool6.tile([128, D], F16, tag="w6")
                nc.sync.dma_start(w_sb[:], wqkv[l, m])
                ps = ps_mm()
                for k in range(KD):
                    nc.tensor.matmul(ps[:], lhsT=w_sb[:, k * 128:(k + 1) * 128],
                                     rhs=xT[:, k, tsl],
                                     start=(k == 0), stop=(k == KD - 1))
                t_sb = qkvpool.tile([128, S], F16, tag="qkv")
                if mi == 0:
                    # Q keeps its bias; K's bias only shifts logits by a
                    # per-query constant, which softmax ignores.
                    nc.scalar.activation(t_sb[:], ps[:], AF.Identity,
                                         bias=bias['bq'][:, m:m + 1])
                else:
                    nc.scalar.activation(t_sb[:], ps[:], AF.Identity)
                tiles[mi] = t_sb
            qt, kt = tiles[0], tiles[1]
            # logits + exp, heads interleaved for PE row-group concurrency
            expP = [expool.tile([128, 4, S], F16, tag="expP", name="expP")
                    for _ in range(2)]
            for st in range(4):
                for e in range(2):
                    po = 64 * e
                    psl = ps_mm()
                    nc.tensor.matmul(
                        psl[:],
                        lhsT=kt[po:po + 64, st * 128:(st + 1) * 128],
                        rhs=qt[po:po + 64, :],
                        start=True, stop=True)
                    nc.scalar.activation(expP[e][:, st, :], psl[:],
                                         AF.Exp, scale=SCALE)
            attn_st[(s, pr)] = expP

        def av_pair(l, s, pr):
            """attn@V with folded denominator + DVE normalize into aT."""
            tsl = slice(s * S, (s + 1) * S)
            expP = attn_st.pop((s, pr))
            p2 = pr % 2
            psA = ps_mm()
            psB = ps_mm()
            for st in range(4):
                nc.tensor.matmul(psA[:], lhsT=vts[p2][0][:, st, :],
                                 rhs=expP[0][:, st, :],
                                 start=(st == 0), stop=(st == 3))
                nc.tensor.matmul(psB[:], lhsT=vts[p2][1][:, st, :],
                                 rhs=expP[1][:, st, :],
                                 start=(st == 0), stop=(st == 3))
            # DVE operands must share base partition, and the reciprocal
            # custom op only works at base 0 (HW rules): hop den0 down via a
            # small SB->SB DMA, reciprocal at base 0, multiply same-base.
            recA = lnp.tile([128, S], F32, tag="rec", name="recA", bufs=2)
            nc.vector.tensor_copy(recA[64:128, :], psA[64:128, :])
            nc.gpsimd.dma_start(recA[0:64, :], recA[64:128, :])
            nc.vector.reciprocal_approx_fast(recA[0:64, :], recA[0:64, :])
            nc.vector.tensor_mul(aT[0:64, pr, tsl], psA[0:64, :], recA[0:64, :])
            recB = lnp.tile([128, S], F32, tag="rec", name="recB", bufs=2)
            nc.vector.reciprocal_approx_fast(recB[0:64, :], psB[0:64, :])
            nc.gpsimd.dma_start(recB[64:128, :], recB[0:64, :])
            nc.vector.tensor_mul(aT[64:128, pr, tsl], psB[64:128, :],
                                 recB[64:128, :])

        def proj_chunk(l, s, bias):
            tsl = slice(s * S, (s + 1) * S)
            for m in range(KD):
                w_sb = wpool6.tile([128, D], F16, tag="w6")
                nc.sync.dma_start(w_sb[:], wproj[l, m])
                ps = ps_mm()
                for k in range(KD):
                    nc.tensor.matmul(ps[:], lhsT=w_sb[:, k * 128:(k + 1) * 128],
                                     rhs=aT[:, k, tsl],
                                     start=(k == 0), stop=(k == KD - 1))
                nc.vector.scalar_tensor_tensor(
                    xT[:, m, tsl], in0=ps[:], scalar=bias['bp'][:, m:m + 1],
                    in1=xT[:, m, tsl], op0=OP.add, op1=OP.add)

        def stats_chunk(s):
            """LN stats matmuls on chunk s. Returns (ps_s, ps_q) PSUM tiles.

            Squares are split across DVE and GpSimd (fp16 out) so the ps_q
            accumulation is not serialized behind one engine."""
            tsl = slice(s * S, (s + 1) * S)
            sqs = []
            for k in range(KD):
                sq = lnp.tile([128, S], F16, tag="sq")
                eng = nc.vector if k % 2 == 0 else nc.gpsimd
                eng.tensor_mul(sq[:], xT[:, k, tsl], xT[:, k, tsl])
                sqs.append(sq)
            ps_s = ps_mm()
            for k in range(KD):
                nc.tensor.matmul(ps_s[:], lhsT=ones_sb[:], rhs=xT[:, k, tsl],
                                 start=(k == 0), stop=(k == KD - 1))
            ps_q = ps_mm()
            for k in range(KD):
                nc.tensor.matmul(ps_q[:], lhsT=ones_sb[:], rhs=sqs[k][:],
                                 start=(k == 0), stop=(k == KD - 1))
            return ps_s, ps_q

        def ln_chain(s, ps_s, ps_q, lw_sb):
            """Center+scale xT chunk s in place (2 passes/k, DVE+gp split)."""
            tsl = slice(s * S, (s + 1) * S)
            mean = lnp.tile([128, S], F16, tag="mean", bufs=2)
            nc.vector.tensor_scalar_mul(mean[:], ps_s[:], 1.0 / D)
            var = lnp.tile([128, S], F32, tag="var", bufs=2)
            nc.vector.tensor_mul(var[:], mean[:], mean[:])
            nc.vector.scalar_tensor_tensor(var[:], in0=ps_q[:], scalar=1.0 / D,
                                           in1=var[:], op0=OP.mult,
                                           op1=OP.subtract)
            nc.scalar.activation(var[:], var[:], AF.Sqrt, bias=eps_sb[:])
            r = lnp.tile([128, S], F32, tag="r", bufs=2)
            nc.vector.reciprocal_approx_fast(r[:], var[:])
            r16 = lnp.tile([128, S], F16, tag="r16", bufs=2)
            nc.vector.tensor_copy(r16[:], r[:])
            for k in range(KD):
                xk = xT[:, k, tsl]
                nc.gpsimd.tensor_sub(xk, xk, mean[:])
                nc.vector.scalar_tensor_tensor(xk, in0=xk,
                                               scalar=lw_sb[:, k:k + 1],
                                               in1=r16[:], op0=OP.mult,
                                               op1=OP.mult)

        def mlp1_chunk(l, s, bias, hooks=None):
            hooks = hooks or {}
            tsl = slice(s * S, (s + 1) * S)
            for m in range(FM):
                w_sb = wpool6.tile([128, D], F16, tag="w6")
                nc.sync.dma_start(w_sb[:], w1[l, m])
                ps = ps_mm()
                for k in range(KD):
                    nc.tensor.matmul(ps[:], lhsT=w_sb[:, k * 128:(k + 1) * 128],
                                     rhs=xT[:, k, tsl],
                                     start=(k == 0), stop=(k == KD - 1))
                nc.scalar.activation(hT[s][:, m, :], ps[:], AF.Gelu,
                                     bias=bias['b1'][:, m:m + 1])
                if m in hooks:
                    hooks[m]()

        def mlp2_chunk(l, s, bias):
            tsl = slice(s * S, (s + 1) * S)
            for m in range(KD):
                w_sb = wpool24.tile([128, F], F16, tag="w24")
                nc.sync.dma_start(w_sb[:], w2[l, m])
                ps = ps_mm()
                for k in range(FM):
                    nc.tensor.matmul(ps[:], lhsT=w_sb[:, k * 128:(k + 1) * 128],
                                     rhs=hT[s][:, k, :],
                                     start=(k == 0), stop=(k == FM - 1))
                nc.vector.scalar_tensor_tensor(
                    xT[:, m, tsl], in0=ps[:], scalar=bias['b2'][:, m:m + 1],
                    in1=xT[:, m, tsl], op0=OP.add, op1=OP.add)

        def attn_block(l, s, bias, hooks=None):
            hooks = hooks or {}
            for pr in range(PAIRS):
                qkv_pair(l, s, pr, bias)
                if pr in hooks:
                    hooks[pr]()
                if pr >= 1:
                    av_pair(l, s, pr - 1)
            av_pair(l, s, PAIRS - 1)

        # ---------------- Pooler (per-seq, so seq0 runs under MLP cover) ----
        poolp = ctx.enter_context(tc.tile_pool(name="poolp", bufs=1))
        bpl_sb = poolp.tile([128, KD], F32)
        nc.sync.dma_start(bpl_sb[:], bpool.rearrange("(m p) -> p m", p=128))
        pool_sb = poolp.tile([128, KD, B_LOC], F32R)
        poolw_sb = [poolp.tile([128, D], F16, name=f"poolw{m}")
                    for m in range(KD)]
        for m in range(KD):
            nc.sync.dma_start(poolw_sb[m][:], wpool[m])

        def pooler_seq(s):
            for m in range(KD):
                ps = ps_mm([128, 1])
                for k in range(KD):
                    nc.tensor.matmul(ps[:], lhsT=poolw_sb[m][:, k * 128:(k + 1) * 128],
                                     rhs=xT[:, k, s * S:s * S + 1],
                                     start=(k == 0), stop=(k == KD - 1))
                nc.scalar.activation(pool_sb[:, m, s:s + 1], ps[:], AF.Tanh,
                                     bias=bpl_sb[:, m:m + 1])

        def pooler_out():
            out_sb = poolp.tile([128, D], F32)
            for k in range(KD):
                pst = psum.tile([128, 128], F32R, tag="mm", name="pst")
                nc.tensor.transpose(pst[:B_LOC, :], pool_sb[:, k, :], ident_sb[:])
                nc.vector.tensor_copy(out_sb[:B_LOC, k * 128:(k + 1) * 128],
                                      pst[:B_LOC, :])
            nc.sync.dma_start(out, out_sb[:B_LOC, :])

        # ---------------- Layers (2-chunk software pipeline) ----------------
        # chunk B's final LN2 of layer l-1 is emitted at the start of layer
        # l's chunk-A attention (one DVE burst per attention block); stats
        # matmuls run right after their producer GEMM stage.
        pending = None  # (ps_s, ps_q, lw) for chunk B's LN2 of prev layer
        for l in range(n_layers):
            bias = load_layer_bias(l)
            if l == 0:
                hooksA = {pr: (lambda tt=tt: embed_tt(tt))
                          for pr, tt in zip(range(4), range(4, 8))}
            else:
                p = pending
                hooksA = {0: (lambda p=p: ln_chain(1, p[0], p[1], p[2]))}
            attn_block(l, 0, bias, hooks=hooksA)
            proj_chunk(l, 0, bias)
            sA = stats_chunk(0)
            attn_block(l, 1, bias,
                       hooks={0: (lambda sA=sA, bias=bias:
                                  ln_chain(0, sA[0], sA[1], bias['l1w']))})
            proj_chunk(l, 1, bias)
            sB = stats_chunk(1)
            mlp1_chunk(l, 0, bias,
                       hooks={2: (lambda sB=sB, bias=bias:
                                  ln_chain(1, sB[0], sB[1], bias['l1w']))})
            mlp2_chunk(l, 0, bias)
            s2A = stats_chunk(0)
            hooksB1 = {2: (lambda s2A=s2A, bias=bias:
                           ln_chain(0, s2A[0], s2A[1], bias['l2w']))}
            if l == n_layers - 1:
                hooksB1[10] = lambda: pooler_seq(0)
            mlp1_chunk(l, 1, bias, hooks=hooksB1)
            mlp2_chunk(l, 1, bias)
            s2B = stats_chunk(1)
            pending = (s2B[0], s2B[1], bias['l2w'])
        ln_chain(1, pending[0], pending[1], pending[2])
        pooler_seq(1)
        pooler_out()




def _prep_host(inputs, n_layers=L):
    f32 = lambda a: np.asarray(a, dtype=np.float32)
    tokens = np.asarray(inputs["tokens"]).astype(np.int32)          # [16, 512]
    possent = f32(inputs["pos_emb"])[0] + f32(inputs["sent_emb"])[0, 0][None, :]

    def tile_stack(w, n):  # w: [L, dout, din]
        n = max(1, n)
        return np.stack([_host_tile_weight(f32(w[i])) for i in range(n)])

    nl = max(1, n_layers)
    # The residual stream xT holds LN output WITHOUT the ln bias. Folds:
    #   qkv_b[l]  += qkv_w[l]  @ prev_ln_b   (emb_ln_b for l=0, ln2_b[l-1] else)
    #   mlp_b1[l] += mlp_w1[l] @ ln1_b[l]
    #   pool_b    += pool_w    @ ln2_b[last]
    #   proj_b[l] += prev_ln_b               (residual-add absorbs the bias)
    #   mlp_b2[l] += ln1_b[l]
    qkv_b = f32(inputs["qkv_b"]).copy()
    proj_b = f32(inputs["proj_b"]).copy()
    mlp_b1 = f32(inputs["mlp_b1"]).copy()
    mlp_b2 = f32(inputs["mlp_b2"]).copy()
    pool_b = f32(inputs["pool_b"]).copy()
    emb_ln_b = f32(inputs["emb_ln_b"])
    ln1_b = f32(inputs["ln1_b"])
    ln2_b = f32(inputs["ln2_b"])
    wvT = np.empty((nl, PAIRS, 128, KD, 128), dtype=np.float16)
    for l in range(nl):
        prev_b = emb_ln_b if l == 0 else ln2_b[l - 1]
        qkv_b[l] = qkv_b[l] + f32(inputs["qkv_w"][l]) @ prev_b
        # the (folded) V bias shifts attn output by a constant: fold into
        # the proj bias instead of applying it on device
        bv = qkv_b[l][2 * D:3 * D].copy()
        qkv_b[l][2 * D:3 * D] = 0.0
        proj_b[l] = proj_b[l] + prev_b + f32(inputs["proj_w"][l]) @ bv
        mlp_b1[l] = mlp_b1[l] + f32(inputs["mlp_w1"][l]) @ ln1_b[l]
        mlp_b2[l] = mlp_b2[l] + ln1_b[l]
        for pr in range(PAIRS):
            vw = f32(inputs["qkv_w"][l])[2 * D + 128 * pr:2 * D + 128 * (pr + 1), :]
            wvT[l, pr] = vw.T.reshape(KD, 128, 128).transpose(1, 0, 2)
    if n_layers >= 1:
        pool_b = pool_b + f32(inputs["pool_w"]) @ ln2_b[nl - 1]
    else:
        pool_b = pool_b + f32(inputs["pool_w"]) @ emb_ln_b
    common = {
        "tok_emb": f32(inputs["tok_emb"]),
        "possent": possent.astype(np.float32),
        "embw": f32(inputs["emb_ln_w"]),
        "wqkv": tile_stack(inputs["qkv_w"], n_layers),
        "wvT": wvT,
        "bqkv": qkv_b[:nl],
        "wproj": tile_stack(inputs["proj_w"], n_layers),
        "bproj": proj_b[:nl],
        "w1": tile_stack(inputs["mlp_w1"], n_layers),
        "b1": mlp_b1[:nl],
        "w2": tile_stack(inputs["mlp_w2"], n_layers),
        "b2": mlp_b2[:nl],
        "ln1w": f32(inputs["ln1_w"])[:nl],
        "ln2w": f32(inputs["ln2_w"])[:nl],
        "wpool": _host_tile_weight(f32(inputs["pool_w"])),
        "bpool": pool_b,
        "ident": np.eye(128, dtype=np.float32),
    }
    in_maps = []
    for c in range(NCORES):
        m = dict(common)
        m["tokens"] = np.ascontiguousarray(
            tokens[c * B_LOC:(c + 1) * B_LOC].reshape(-1))
        in_maps.append(m)
    return in_maps


def kernel(**inputs) -> np.ndarray:
    global _CACHED_NC
    if _CACHED_NC is None:
        _CACHED_NC = build_nc(L)
    in_maps = _prep_host(inputs, L)
    res = run_bass_kernel_spmd(_CACHED_NC, in_maps,
                               core_ids=list(range(NCORES)), trace=False)
    return np.concatenate([res.results[c]["out"] for c in range(NCORES)], axis=0)



# revision 32
# speedup vs baseline: 1.0928x; 1.0326x over previous
"""BERT-base forward (B=16, S=512, D=768, H=12, L=12) on 8 Trainium2 NeuronCores.

Sharding: data-parallel over batch — each core runs 2 sequences (1024 tokens)
with a full replica of the weights. No collectives.

Device layout: "transposed activations" — activations live as x^T [D, tokens]
(features on SBUF partitions, tokens on the free dim), so every linear layer is
out^T = W^T.T @ x^T with the (host-pre-tiled) weight as the stationary operand
and 512-token chunks as the moving operand.

Precision (v3): fp16 weights and fp16 residual stream, used directly as the
GEMM moving operand (no bf16 shadow copy), fp32 PSUM accumulation. fp16 LN
mean/r operands enable 2x DVE perf modes; Q/K PSUM->SBUF copies moved to the
scalar engine.

Key structure (v2, rewritten for PE occupancy):
- Single fp32r residual stream xT; no bf16 shadow. LayerNorm biases are folded
  into downstream GEMM biases (host) and into the residual-add bias (host), so
  the device LN is only center+scale: 2 elementwise passes, split DVE/GpSimd.
- The two 512-token chunks (= the 2 sequences) are software-pipelined through
  the layer stages so each chunk's LN / softmax chains overlap the other
  chunk's GEMMs.
- Softmax denominator is folded into the attn@V matmul: the stationary is
  [V_e^T | ones] so u and den come out of one PSUM; normalize is 2 DVE ops.
- QKV bias+copy moved from ACT to DVE (tensor_scalar from PSUM).
- V^T tiles produced by rectangular DMA transposes spread across the sync and
  scalar queues, issued right after the V GEMM of each pair.
- QK^T logits for the two heads of a pair are issued interleaved so they run
  on disjoint PE row-groups (contraction=64) concurrently.
"""
import sys
sys.path.insert(0, '/opt/trn_rl_repo')

import numpy as np
import ml_dtypes
import concourse.bass as bass
import concourse.tile as tile
from concourse import bacc, mybir
from concourse.bass_utils import run_bass_kernel_spmd

# Model shapes (hardcoded)
V = 30522
S = 512
D = 768
H = 12
L = 12
F = 3072
B = 16
HD = 64
EPS = 1e-12
SCALE = HD ** (-0.5)

NCORES = 8
B_LOC = B // NCORES          # 2 sequences per core
T = B_LOC * S                # 1024 tokens per core
KD = D // 128                # 6 k-tiles over D
QKVM = 3 * D // 128          # 18 m-tiles for qkv
FM = F // 128                # 24 m-tiles over mlp hidden
PAIRS = H // 2               # 6 head-pairs
NT = B_LOC                   # 2 chunks = the 2 sequences

F32 = mybir.dt.float32
F32R = mybir.dt.float32r
BF16 = mybir.dt.bfloat16
F16 = mybir.dt.float16
I32 = mybir.dt.int32
AF = mybir.ActivationFunctionType
OP = mybir.AluOpType

_CACHED_NC = None


def _host_tile_weight(w_t):
    """w_t: [dout, din] torch-Linear weight. Returns [m_tiles, 128, din] where
    slice [m] is (w_t.T)[:, m*128:(m+1)*128] laid out partition-major."""
    dout, din = w_t.shape
    m_tiles, k_tiles = dout // 128, din // 128
    a = np.ascontiguousarray(w_t.T)                      # [din, dout]
    a = a.reshape(k_tiles, 128, m_tiles, 128)            # [k, p, m, w]
    a = np.ascontiguousarray(a.transpose(2, 1, 0, 3))    # [m, p, k, w]
    return a.reshape(m_tiles, 128, din).astype(np.float16)


def build_nc(n_layers=L):
    nc = bacc.Bacc("TRN2", target_bir_lowering=False, debug=False)

    def din(name, shape, dt=F16):
        return nc.dram_tensor(name, shape, dt, kind="ExternalInput").ap()

    nl = max(1, n_layers)
    tokens = din("tokens", [T], I32)
    tok_emb = din("tok_emb", [V, D], F32R)
    possent = din("possent", [S, D], F32)
    embw = din("embw", [D], F32)
    wqkv = din("wqkv", [nl, QKVM, 128, D])
    wvT = din("wvT", [nl, PAIRS, 128, KD, 128])
    bqkv = din("bqkv", [nl, 3 * D], F32)
    wproj = din("wproj", [nl, KD, 128, D])
    bproj = din("bproj", [nl, D], F32)          # pre-folded: + prev ln bias
    w1 = din("w1", [nl, FM, 128, D])
    b1 = din("b1", [nl, F], F32)
    w2 = din("w2", [nl, KD, 128, F])
    b2 = din("b2", [nl, D], F32)                # pre-folded: + ln1 bias
    ln1w = din("ln1w", [nl, D], F32)
    ln2w = din("ln2w", [nl, D], F32)
    wpool = din("wpool", [KD, 128, D], F16)
    bpool = din("bpool", [D], F32)
    ident = din("ident", [128, 128], F32R)
    out = nc.dram_tensor("out", [B_LOC, D], F32, kind="ExternalOutput").ap()

    with tile.TileContext(nc) as tc:
        _build_body(nc, tc, n_layers, tokens, tok_emb, possent, embw,
                    wqkv, wvT, bqkv, wproj, bproj, w1, b1, w2, b2,
                    ln1w, ln2w, wpool, bpool, ident, out)
    nc.compile()
    return nc


def _build_body(nc, tc, n_layers, tokens, tok_emb, possent, embw,
                wqkv, wvT, bqkv, wproj, bproj, w1, b1, w2, b2,
                ln1w, ln2w, wpool, bpool, ident, out):
    from contextlib import ExitStack
    ctx = ExitStack()
    with ctx:
        consts = ctx.enter_context(tc.tile_pool(name="consts", bufs=1))
        xpool = ctx.enter_context(tc.tile_pool(name="xpool", bufs=1))
        qkvpool = ctx.enter_context(tc.tile_pool(name="qkvpool", bufs=6))
        expool = ctx.enter_context(tc.tile_pool(name="expool", bufs=4))
        wpool6 = ctx.enter_context(tc.tile_pool(name="wpool6", bufs=8))
        wpool24 = ctx.enter_context(tc.tile_pool(name="wpool24", bufs=2))
        biasp = ctx.enter_context(tc.tile_pool(name="biasp", bufs=2))
        lnp = ctx.enter_context(tc.tile_pool(name="lnp", bufs=6))
        psum = ctx.enter_context(tc.tile_pool(name="psum", bufs=8, space="PSUM"))

        ident_sb = consts.tile([128, 128], F32R)
        nc.sync.dma_start(ident_sb[:], ident)
        ones_sb = consts.tile([128, 128], F16)
        nc.gpsimd.memset(ones_sb[:], 1.0)
        eps_sb = consts.tile([128, 1], F32)
        nc.vector.memset(eps_sb[:], EPS)

        # residual stream x^T (fp16: used directly as the GEMM moving operand)
        xT = xpool.tile([128, KD, T], F16)
        # attention output a^T (fp16: proj GEMM input)
        aT = xpool.tile([128, KD, T], F16)
        # MLP hidden per chunk (fp16)
        hT = [xpool.tile([128, FM, S], F16, name=f"hT{s}") for s in range(NT)]
        # static [V_e^T | ones] stationaries: [parity][e] -> [128, 4, 128]
        # ones half initialized once; transposes overwrite only the V half.
        vts = [[consts.tile([128, 4, 128], F16, name=f"vts{p}{e}")
                for e in range(2)] for p in range(2)]
        for p in range(2):
            nc.gpsimd.memset(vts[p][0][:, :, 64:128], 1.0)
            nc.gpsimd.memset(vts[p][1][:, :, 0:64], 1.0)

        def ps_mm(shape=None, tag="mm"):
            return psum.tile(shape or [128, S], F32, tag="mm", name="ps")

        # ---------------- Embedding ----------------
        embp = ctx.enter_context(tc.tile_pool(name="embp", bufs=4))
        embw_sb = embp.tile([128, D], F32, tag="embw", bufs=1)
        nc.sync.dma_start(embw_sb[:], embw[None, :].to_broadcast([128, D]))

        def embed_tt(tt):
            idx_sb = embp.tile([128, 1], I32, tag="idx")
            nc.sync.dma_start(idx_sb[:], tokens[tt * 128:(tt + 1) * 128, None])
            g_sb = embp.tile([128, D], F32R, tag="g")
            nc.gpsimd.indirect_dma_start(
                out=g_sb[:], out_offset=None, in_=tok_emb,
                in_offset=bass.IndirectOffsetOnAxis(ap=idx_sb[:, :1], axis=0))
            p_sb = embp.tile([128, D], F32, tag="p")
            prow = (tt * 128) % S
            nc.sync.dma_start(p_sb[:], possent[prow:prow + 128, :])
            nc.vector.tensor_add(g_sb[:], g_sb[:], p_sb[:])
            # LayerNorm over free dim (d): bn_stats in 2 subgroups of 384
            st_sb = embp.tile([128, 2, 6], F32, tag="st")
            gv = g_sb[:].rearrange("p (a b) -> p a b", a=2)
            for a in range(2):
                nc.vector.bn_stats(st_sb[:, a, :], gv[:, a, :])
            mv = embp.tile([128, 2], F32, tag="mv")
            nc.vector.bn_aggr(mv[:], st_sb[:])
            sd = embp.tile([128, 1], F32, tag="sd")
            nc.scalar.activation(sd[:], mv[:, 1:2], AF.Sqrt, bias=eps_sb[:])
            nc.vector.reciprocal_approx_fast(sd[:], sd[:])
            nc.vector.tensor_scalar(g_sb[:], g_sb[:], mv[:, 0:1], sd[:],
                                    op0=OP.subtract, op1=OP.mult)
            nc.vector.tensor_mul(g_sb[:], g_sb[:], embw_sb[:])
            # transpose into xT (no ln bias: folded into l=0 qkv bias and
            # the l=0 proj residual-add bias on the host)
            for k in range(KD):
                pst = psum.tile([128, 128], F32R, tag="mm", name="pst")
                nc.tensor.transpose(pst[:], g_sb[:, k * 128:(k + 1) * 128],
                                    ident_sb[:])
                ttsl = slice(tt * 128, (tt + 1) * 128)
                nc.vector.tensor_copy(xT[:, k, ttsl], pst[:])

        # chunk A now; chunk B (tt 4..7) deferred into layer-0 attention hooks
        for tt in range(4):
            embed_tt(tt)

        # ---------------- Per-layer helpers ----------------
        def load_layer_bias(l):
            bq_sb = biasp.tile([128, QKVM], F32, tag="bq")
            nc.sync.dma_start(bq_sb[:], bqkv[l].rearrange("(m p) -> p m", p=128))
            bp_sb = biasp.tile([128, KD], F32, tag="bp")
            nc.sync.dma_start(bp_sb[:], bproj[l].rearrange("(m p) -> p m", p=128))
            b1_sb = biasp.tile([128, FM], F32, tag="b1")
            nc.sync.dma_start(b1_sb[:], b1[l].rearrange("(m p) -> p m", p=128))
            b2_sb = biasp.tile([128, KD], F32, tag="b2")
            nc.sync.dma_start(b2_sb[:], b2[l].rearrange("(m p) -> p m", p=128))
            l1w_sb = biasp.tile([128, KD], F32, tag="l1w")
            nc.sync.dma_start(l1w_sb[:], ln1w[l].rearrange("(k p) -> p k", p=128))
            l2w_sb = biasp.tile([128, KD], F32, tag="l2w")
            nc.sync.dma_start(l2w_sb[:], ln2w[l].rearrange("(k p) -> p k", p=128))
            return dict(bq=bq_sb, bp=bp_sb, b1=b1_sb, b2=b2_sb,
                        l1w=l1w_sb, l2w=l2w_sb)

        # per-(chunk, pair) state: q/k tiles, then expP tiles
        qk_st = {}
        attn_st = {}

        def qkv_gemms(l, s, pr, bias):
            """V^T, K, Q GEMMs for pair pr of chunk s (copies on DVE/gp).

            V is computed directly in [token, feature] layout: stationary =
            xT token-slices, moving = Wv^T k-tiles, so no transposes are
            needed. The V bias is folded into the proj bias on the host
            (softmax rows sum to 1, so it shifts attn output by a constant).
            """
            tsl = slice(s * S, (s + 1) * S)
            p2 = pr % 2
            wv_sb = wpool6.tile([128, D], F16, tag="w6", name="wv_sb")
            nc.sync.dma_start(wv_sb[:], wvT[l, pr].rearrange("p k f -> p (k f)"))
            wvv = wv_sb[:].rearrange("p (k f) -> p k f", k=KD)
            for st in range(4):
                ksl = slice(s * S + st * 128, s * S + (st + 1) * 128)
                psv = ps_mm([128, 128], tag="psv")
                for k in range(KD):
                    nc.tensor.matmul(psv[:], lhsT=xT[:, k, ksl],
                                     rhs=wvv[:, k, :],
                                     start=(k == 0), stop=(k == KD - 1))
                nc.vector.tensor_copy(vts[p2][0][:, st, 0:64], psv[:, 0:64])
                nc.vector.tensor_copy(vts[p2][1][:, st, 64:128], psv[:, 64:128])
            tiles = {}
            for mi, m in ((1, 6 + pr), (0, pr)):
                w_sb = wpool6.tile([128, D], F16, tag="w6")
                nc.sync.dma_start(w_sb[:], wqkv[l, m])
                ps = ps_mm()
                for k in range(KD):
                    nc.tensor.matmul(ps[:], lhsT=w_sb[:, k * 128:(k + 1) * 128],
                                     rhs=xT[:, k, tsl],
                                     start=(k == 0), stop=(k == KD - 1))
                t_sb = qkvpool.tile([128, S], F16, tag="qkv")
                if mi == 0:
                    # Q keeps its bias; K's bias only shifts logits by a
                    # per-query constant, which softmax ignores.
                    nc.vector.tensor_scalar(t_sb[:], ps[:],
                                            bias['bq'][:, m:m + 1],
                                            None, op0=OP.add)
                else:
                    # K copy on the scalar engine (DVE is loaded with the
                    # vts copies + av normalize; ACT has exp headroom only)
                    nc.scalar.activation(t_sb[:], ps[:], AF.Identity)
                tiles[mi] = t_sb
            qk_st[(s, pr)] = (tiles[0], tiles[1])

        def logits_exp(s, pr):
            qt, kt = qk_st.pop((s, pr))
            # logits + exp, heads interleaved for PE row-group concurrency
            expP = [expool.tile([128, 4, S], F16, tag="expP", name="expP")
                    for _ in range(2)]
            for st in range(4):
                for e in range(2):
                    po = 64 * e
                    psl = ps_mm()
                    nc.tensor.matmul(
                        psl[:],
                        lhsT=kt[po:po + 64, st * 128:(st + 1) * 128],
                        rhs=qt[po:po + 64, :],
                        start=True, stop=True)
                    nc.scalar.activation(expP[e][:, st, :], psl[:],
                                         AF.Exp, scale=SCALE)
            attn_st[(s, pr)] = expP

        def av_pair(l, s, pr):
            """attn@V with folded denominator + DVE normalize into aT."""
            tsl = slice(s * S, (s + 1) * S)
            expP = attn_st.pop((s, pr))
            p2 = pr % 2
            psA = ps_mm()
            psB = ps_mm()
            for st in range(4):
                nc.tensor.matmul(psA[:], lhsT=vts[p2][0][:, st, :],
                                 rhs=expP[0][:, st, :],
                                 start=(st == 0), stop=(st == 3))
                nc.tensor.matmul(psB[:], lhsT=vts[p2][1][:, st, :],
                                 rhs=expP[1][:, st, :],
                                 start=(st == 0), stop=(st == 3))
            # DVE operands must share base partition, and the reciprocal
            # custom op only works at base 0 (HW rules): hop den0 down via a
            # small SB->SB DMA, reciprocal at base 0, multiply same-base.
            recA = lnp.tile([128, S], F32, tag="rec", name="recA", bufs=2)
            nc.vector.tensor_copy(recA[64:128, :], psA[64:128, :])
            nc.sync.dma_start(recA[0:64, :], recA[64:128, :])
            nc.vector.reciprocal_approx_fast(recA[0:64, :], recA[0:64, :])
            nc.vector.tensor_mul(aT[0:64, pr, tsl], psA[0:64, :], recA[0:64, :])
            recB = lnp.tile([128, S], F32, tag="rec", name="recB", bufs=2)
            nc.vector.reciprocal_approx_fast(recB[0:64, :], psB[0:64, :])
            nc.sync.dma_start(recB[64:128, :], recB[0:64, :])
            nc.vector.tensor_mul(aT[64:128, pr, tsl], psB[64:128, :],
                                 recB[64:128, :])

        def proj_chunk(l, s, bias):
            tsl = slice(s * S, (s + 1) * S)
            for m in range(KD):
                w_sb = wpool6.tile([128, D], F16, tag="w6")
                nc.sync.dma_start(w_sb[:], wproj[l, m])
                ps = ps_mm()
                for k in range(KD):
                    nc.tensor.matmul(ps[:], lhsT=w_sb[:, k * 128:(k + 1) * 128],
                                     rhs=aT[:, k, tsl],
                                     start=(k == 0), stop=(k == KD - 1))
                nc.vector.scalar_tensor_tensor(
                    xT[:, m, tsl], in0=ps[:], scalar=bias['bp'][:, m:m + 1],
                    in1=xT[:, m, tsl], op0=OP.add, op1=OP.add)

        def stats_chunk(s):
            """LN stats matmuls on chunk s. Returns (ps_s, ps_q) PSUM tiles.

            Squares are split across DVE and GpSimd (fp16 out) so the ps_q
            accumulation is not serialized behind one engine."""
            tsl = slice(s * S, (s + 1) * S)
            sqs = []
            for k in range(KD):
                sq = lnp.tile([128, S], F16, tag="sq")
                eng = nc.vector if k % 2 == 0 else nc.gpsimd
                eng.tensor_mul(sq[:], xT[:, k, tsl], xT[:, k, tsl])
                sqs.append(sq)
            ps_s = ps_mm()
            for k in range(KD):
                nc.tensor.matmul(ps_s[:], lhsT=ones_sb[:], rhs=xT[:, k, tsl],
                                 start=(k == 0), stop=(k == KD - 1))
            ps_q = ps_mm()
            for k in range(KD):
                nc.tensor.matmul(ps_q[:], lhsT=ones_sb[:], rhs=sqs[k][:],
                                 start=(k == 0), stop=(k == KD - 1))
            return ps_s, ps_q

        def ln_parts(s, ps_s, ps_q, lw_sb):
            """LN of xT chunk s split into 4 closures: stats prefix + 3
            two-k-tile center+scale parts (to spread across attn hooks)."""
            tsl = slice(s * S, (s + 1) * S)
            st = {}

            def prefix():
                mean = lnp.tile([128, S], F16, tag="mean", bufs=2)
                nc.vector.tensor_scalar_mul(mean[:], ps_s[:], 1.0 / D)
                var = lnp.tile([128, S], F32, tag="var", bufs=2)
                nc.vector.tensor_mul(var[:], mean[:], mean[:])
                nc.vector.scalar_tensor_tensor(var[:], in0=ps_q[:],
                                               scalar=1.0 / D, in1=var[:],
                                               op0=OP.mult, op1=OP.subtract)
                nc.scalar.activation(var[:], var[:], AF.Sqrt, bias=eps_sb[:])
                r = lnp.tile([128, S], F32, tag="r", bufs=2)
                nc.vector.reciprocal_approx_fast(r[:], var[:])
                r16 = lnp.tile([128, S], F16, tag="r16", bufs=2)
                nc.vector.tensor_copy(r16[:], r[:])
                st['mean'], st['r16'] = mean, r16

            def kpart(k0):
                def f():
                    for k in (k0, k0 + 1):
                        xk = xT[:, k, tsl]
                        nc.gpsimd.tensor_sub(xk, xk, st['mean'][:])
                        nc.vector.scalar_tensor_tensor(
                            xk, in0=xk, scalar=lw_sb[:, k:k + 1],
                            in1=st['r16'][:], op0=OP.mult, op1=OP.mult)
                return f

            return [prefix, kpart(0), kpart(2), kpart(4)]

        def ln_chain(s, ps_s, ps_q, lw_sb):
            """Center+scale xT chunk s in place (whole chain, for mlp hooks)."""
            for part in ln_parts(s, ps_s, ps_q, lw_sb):
                part()

        def mlp1_chunk(l, s, bias, hooks=None):
            hooks = hooks or {}
            tsl = slice(s * S, (s + 1) * S)
            for m in range(FM):
                w_sb = wpool6.tile([128, D], F16, tag="w6")
                nc.sync.dma_start(w_sb[:], w1[l, m])
                ps = ps_mm()
                for k in range(KD):
                    nc.tensor.matmul(ps[:], lhsT=w_sb[:, k * 128:(k + 1) * 128],
                                     rhs=xT[:, k, tsl],
                                     start=(k == 0), stop=(k == KD - 1))
                nc.scalar.activation(hT[s][:, m, :], ps[:], AF.Gelu,
                                     bias=bias['b1'][:, m:m + 1])
                if m in hooks:
                    hooks[m]()

        def mlp2_chunk(l, s, bias):
            tsl = slice(s * S, (s + 1) * S)
            for m in range(KD):
                w_sb = wpool24.tile([128, F], F16, tag="w24")
                nc.sync.dma_start(w_sb[:], w2[l, m])
                ps = ps_mm()
                for k in range(FM):
                    nc.tensor.matmul(ps[:], lhsT=w_sb[:, k * 128:(k + 1) * 128],
                                     rhs=hT[s][:, k, :],
                                     start=(k == 0), stop=(k == FM - 1))
                nc.vector.scalar_tensor_tensor(
                    xT[:, m, tsl], in0=ps[:], scalar=bias['b2'][:, m:m + 1],
                    in1=xT[:, m, tsl], op0=OP.add, op1=OP.add)

        def attn_block(l, s, bias, hooks=None):
            # av_pair(pr-1) is issued between pair pr's QKV GEMMs and its
            # logits so the PE covers the Q/K PSUM->SBUF copy latency.
            hooks = hooks or {}
            for pr in range(PAIRS):
                qkv_gemms(l, s, pr, bias)
                if pr >= 1:
                    av_pair(l, s, pr - 1)
                logits_exp(s, pr)
                if pr in hooks:
                    hooks[pr]()
            av_pair(l, s, PAIRS - 1)

        # ---------------- Pooler (per-seq, so seq0 runs under MLP cover) ----
        # ln2w of the last layer is folded into wpool on the host, so the
        # final LN reduces to center+scale of the single pooled token column.
        poolp = ctx.enter_context(tc.tile_pool(name="poolp", bufs=1))
        bpl_sb = poolp.tile([128, KD], F32)
        nc.sync.dma_start(bpl_sb[:], bpool.rearrange("(m p) -> p m", p=128))
        pool_sb = poolp.tile([128, KD, B_LOC], F32R)
        poolx = poolp.tile([128, KD, B_LOC], F16)
        poolw_sb = [poolp.tile([128, D], F16, name=f"poolw{m}")
                    for m in range(KD)]
        for m in range(KD):
            nc.sync.dma_start(poolw_sb[m][:], wpool[m])

        def stats_token(s):
            """LN stats matmuls over the first-token column of seq s only."""
            col = slice(s * S, s * S + 1)
            sqp = lnp.tile([128, KD, 1], F16, tag="sqp")
            nc.vector.tensor_mul(sqp[:], xT[:, :, col], xT[:, :, col])
            ps_s = ps_mm([128, 1])
            for k in range(KD):
                nc.tensor.matmul(ps_s[:], lhsT=ones_sb[:], rhs=xT[:, k, col],
                                 start=(k == 0), stop=(k == KD - 1))
            ps_q = ps_mm([128, 1])
            for k in range(KD):
                nc.tensor.matmul(ps_q[:], lhsT=ones_sb[:], rhs=sqp[:, k, :],
                                 start=(k == 0), stop=(k == KD - 1))
            return ps_s, ps_q

        def pool_prep(s, stats):
            """Mini LN chain for the pooled token of seq s -> poolx."""
            ps_s, ps_q = stats
            m0 = poolp.tile([128, 1], F32, name=f"pm{s}")
            nc.vector.tensor_scalar_mul(m0[:], ps_s[:], 1.0 / D)
            v0 = poolp.tile([128, 1], F32, name=f"pv{s}")
            nc.vector.tensor_mul(v0[:], m0[:], m0[:])
            nc.vector.scalar_tensor_tensor(v0[:], in0=ps_q[:], scalar=1.0 / D,
                                           in1=v0[:], op0=OP.mult,
                                           op1=OP.subtract)
            nc.scalar.activation(v0[:], v0[:], AF.Sqrt, bias=eps_sb[:])
            nc.vector.reciprocal_approx_fast(v0[:], v0[:])
            nc.vector.tensor_scalar(poolx[:, :, s:s + 1],
                                    xT[:, :, s * S:s * S + 1], m0[:], v0[:],
                                    op0=OP.subtract, op1=OP.mult)

        def pooler_seq(s):
            for m in range(KD):
                ps = ps_mm([128, 1])
                for k in range(KD):
                    nc.tensor.matmul(ps[:], lhsT=poolw_sb[m][:, k * 128:(k + 1) * 128],
                                     rhs=poolx[:, k, s:s + 1],
                                     start=(k == 0), stop=(k == KD - 1))
                nc.scalar.activation(pool_sb[:, m, s:s + 1], ps[:], AF.Tanh,
                                     bias=bpl_sb[:, m:m + 1])

        def pooler_out():
            out_sb = poolp.tile([128, D], F32)
            for k in range(KD):
                pst = psum.tile([128, 128], F32R, tag="mm", name="pst")
                nc.tensor.transpose(pst[:B_LOC, :], pool_sb[:, k, :], ident_sb[:])
                nc.vector.tensor_copy(out_sb[:B_LOC, k * 128:(k + 1) * 128],
                                      pst[:B_LOC, :])
            nc.sync.dma_start(out, out_sb[:B_LOC, :])

        # ---------------- Layers (2-chunk software pipeline) ----------------
        # chunk B's final LN2 of layer l-1 is spread over the first 4 pairs
        # of layer l's chunk-A attention (prefix + 3 two-k-tile parts so no
        # single pair's DVE/gp queue overruns the PE pair time); stats
        # matmuls run right after their producer GEMM stage.
        pending = None  # (ps_s, ps_q, lw) for chunk B's LN2 of prev layer
        for l in range(n_layers):
            bias = load_layer_bias(l)
            if l == 0:
                hooksA = {pr: (lambda tt=tt: embed_tt(tt))
                          for pr, tt in zip(range(4), range(4, 8))}
            else:
                hooksA = dict(enumerate(ln_parts(1, *pending)))
            attn_block(l, 0, bias, hooks=hooksA)
            proj_chunk(l, 0, bias)
            sA = stats_chunk(0)
            attn_block(l, 1, bias,
                       hooks=dict(enumerate(
                           ln_parts(0, sA[0], sA[1], bias['l1w']))))
            proj_chunk(l, 1, bias)
            sB = stats_chunk(1)
            mlp1_chunk(l, 0, bias,
                       hooks={2: (lambda sB=sB, bias=bias:
                                  ln_chain(1, sB[0], sB[1], bias['l1w']))})
            mlp2_chunk(l, 0, bias)
            if l < n_layers - 1:
                s2A = stats_chunk(0)
                hooksB1 = {2: (lambda s2A=s2A, bias=bias:
                               ln_chain(0, s2A[0], s2A[1], bias['l2w']))}
            else:
                spA = stats_token(0)
                hooksB1 = {2: (lambda spA=spA: pool_prep(0, spA)),
                           10: (lambda: pooler_seq(0))}
            mlp1_chunk(l, 1, bias, hooks=hooksB1)
            mlp2_chunk(l, 1, bias)
            if l < n_layers - 1:
                s2B = stats_chunk(1)
                pending = (s2B[0], s2B[1], bias['l2w'])
        spB = stats_token(1)
        pool_prep(1, spB)
        pooler_seq(1)
        pooler_out()




def _prep_host(inputs, n_layers=L):
    f32 = lambda a: np.asarray(a, dtype=np.float32)
    tokens = np.asarray(inputs["tokens"]).astype(np.int32)          # [16, 512]
    possent = f32(inputs["pos_emb"])[0] + f32(inputs["sent_emb"])[0, 0][None, :]

    def tile_stack(w, n):  # w: [L, dout, din]
        n = max(1, n)
        return np.stack([_host_tile_weight(f32(w[i])) for i in range(n)])

    nl = max(1, n_layers)
    # The residual stream xT holds LN output WITHOUT the ln bias. Folds:
    #   qkv_b[l]  += qkv_w[l]  @ prev_ln_b   (emb_ln_b for l=0, ln2_b[l-1] else)
    #   mlp_b1[l] += mlp_w1[l] @ ln1_b[l]
    #   pool_b    += pool_w    @ ln2_b[last]
    #   proj_b[l] += prev_ln_b               (residual-add absorbs the bias)
    #   mlp_b2[l] += ln1_b[l]
    qkv_b = f32(inputs["qkv_b"]).copy()
    proj_b = f32(inputs["proj_b"]).copy()
    mlp_b1 = f32(inputs["mlp_b1"]).copy()
    mlp_b2 = f32(inputs["mlp_b2"]).copy()
    pool_b = f32(inputs["pool_b"]).copy()
    emb_ln_b = f32(inputs["emb_ln_b"])
    ln1_b = f32(inputs["ln1_b"])
    ln2_b = f32(inputs["ln2_b"])
    wvT = np.empty((nl, PAIRS, 128, KD, 128), dtype=np.float16)
    for l in range(nl):
        prev_b = emb_ln_b if l == 0 else ln2_b[l - 1]
        qkv_b[l] = qkv_b[l] + f32(inputs["qkv_w"][l]) @ prev_b
        # the (folded) V bias shifts attn output by a constant: fold into
        # the proj bias instead of applying it on device
        bv = qkv_b[l][2 * D:3 * D].copy()
        qkv_b[l][2 * D:3 * D] = 0.0
        proj_b[l] = proj_b[l] + prev_b + f32(inputs["proj_w"][l]) @ bv
        mlp_b1[l] = mlp_b1[l] + f32(inputs["mlp_w1"][l]) @ ln1_b[l]
        mlp_b2[l] = mlp_b2[l] + ln1_b[l]
        for pr in range(PAIRS):
            vw = f32(inputs["qkv_w"][l])[2 * D + 128 * pr:2 * D + 128 * (pr + 1), :]
            wvT[l, pr] = vw.T.reshape(KD, 128, 128).transpose(1, 0, 2)
    if n_layers >= 1:
        pool_b = pool_b + f32(inputs["pool_w"]) @ ln2_b[nl - 1]
        last_lnw = f32(inputs["ln2_w"])[nl - 1]
    else:
        pool_b = pool_b + f32(inputs["pool_w"]) @ emb_ln_b
        last_lnw = f32(inputs["emb_ln_w"])
    # device-side final LN is center+scale only: fold ln weight into wpool
    wpool_eff = f32(inputs["pool_w"]) * last_lnw[None, :]
    common = {
        "tok_emb": f32(inputs["tok_emb"]),
        "possent": possent.astype(np.float32),
        "embw": f32(inputs["emb_ln_w"]),
        "wqkv": tile_stack(inputs["qkv_w"], n_layers),
        "wvT": wvT,
        "bqkv": qkv_b[:nl],
        "wproj": tile_stack(inputs["proj_w"], n_layers),
        "bproj": proj_b[:nl],
        "w1": tile_stack(inputs["mlp_w1"], n_layers),
        "b1": mlp_b1[:nl],
        "w2": tile_stack(inputs["mlp_w2"], n_layers),
        "b2": mlp_b2[:nl],
        "ln1w": f32(inputs["ln1_w"])[:nl],
        "ln2w": f32(inputs["ln2_w"])[:nl],
        "wpool": _host_tile_weight(wpool_eff),
        "bpool": pool_b,
        "ident": np.eye(128, dtype=np.float32),
    }
    in_maps = []
    for c in range(NCORES):
        m = dict(common)
        m["tokens"] = np.ascontiguousarray(
            tokens[c * B_LOC:(c + 1) * B_LOC].reshape(-1))
        in_maps.append(m)
    return in_maps


def kernel(**inputs) -> np.ndarray:
    global _CACHED_NC
    if _CACHED_NC is None:
        _CACHED_NC = build_nc(L)
    in_maps = _prep_host(inputs, L)
    res = run_bass_kernel_spmd(_CACHED_NC, in_maps,
                               core_ids=list(range(NCORES)), trace=False)
    return np.concatenate([res.results[c]["out"] for c in range(NCORES)], axis=0)

